# revision 75
# baseline (speedup 1.0000x reference)
"""Trainium2 Bass kernel for MockGCN segment-reduce problem.

Pipeline (per 8-way data-parallel shard, graphs grouped per shard):
  h1 = relu(x @ W_in + b_in)         [N, 64]
  h2 = relu(h1 @ W_h + b_h)          [N, 64]
  pooled[g] = mean_{i in g} h2[i]    [G, 64]
  out = pooled @ W_out + b_out       [G, 5]

Device layout: features-on-partitions ("T orientation"), nodes 2-packed
across the 128 partitions (64 feats x 2 node streams) and 4-interleaved
along the free axis so a column of the partial tensor holds 4 consecutive
nodes.  The host pads every segment to a multiple of 4 nodes, packs
x into xT_dev [128, C4] bf16, and the device emits per-node h2 values as
fp8e4 [128, 2*C4] (unfolded: same output bytes as a folded bf16 tensor,
but no on-device fold work).  The host folds streams/partitions, does
the per-segment combine, mean division, pad correction, and the tiny
[G,64]@[64,5] matmul.

Stage map (per 512-col quantum = 2048 nodes):
  DMA in xT chunk [128, 1024] bf16 (2 quanta per dma_start)
  PE:  MM-A (rows 0-63, row-tiled)  -> psum ab[:, 0:512]    (h1raw stream0)
       MM-B (rows 64-127, row-tiled)-> psum ab[:, 512:1024] (h1raw stream1)
  D1:  relu(ab + b1) -> rhs2 [128,1024] bf16 SBUF  (ScalarE)
  PE:  MM x2 (K=128, blkdiag(W2,W2)) -> psum cd[:, 0:512], cd[:, 512:1024]
  D2:  relu(cd + b2) -> h2r [128,1024] fp8e4 SBUF  (VectorE; ScalarE
       steals the whole tile every K_STEAL-th quantum to balance)
  DMA out h2r -> pout[:, 2*q*512 : ...] (2 quanta per dma_start)

The PSUM->SBUF relu drains are the hard wall: on TRN2 only ScalarE
(1.2 col/ns) and VectorE (0.96 col/ns, no 2x modes from fp32 PSUM) can
read PSUM -- GpSimd compute and DMA cannot touch it -- and 2048 psum
columns per 2048 nodes must pass through them.  Matmuls must write fp32
to PSUM (16-bit PSUM is TRN3+), so the drains cannot use VectorE's
2-byte fast paths.  Stage-2 emission trails stage-1 by two quanta so
the PE's in-order stream never blocks ScalarE's D1 cadence, and the
first x batch is split so quantum 0 starts one transfer earlier.
"""

import sys

if "/opt/trn_rl_repo" not in sys.path:
    sys.path.insert(0, "/opt/trn_rl_repo")

from contextlib import ExitStack

import ml_dtypes
import numpy as np

N_CORES = 8
G_TOTAL = 8192
F_IN = 32
H_DIM = 64
Q_COLS = 512  # packed node columns per quantum
PACK = 4  # nodes per packed column
DMA_BATCH = 2  # quanta per input DMA transfer
OUT_BATCH = 2  # quanta per output DMA transfer
K_STEAL = 10  # every k-th quantum ScalarE takes the whole D2 (balances engines)
S_TAIL = 2 * Q_COLS  # stolen D2 columns (2*Q_COLS = whole-tile steal)
S_PHASE = 8  # steal when p % K_STEAL == S_PHASE
END_ALT = 2  # over the last END_ALT quanta, alternate D2 between ACT/DVE
RAMP_SWAP = 0  # odd quanta < RAMP_SWAP get their D1 on VectorE (idle at ramp)
POST_SPLIT = False  # drain the post-steal D2 in halves (measured worse)
TAIL_SINGLE = 2  # final quanta written one-per-DMA (earlier, smaller last DMA)
PE_WARM = False  # dummy matmul at t~0 to start the PE p-state ramp (no effect)
RAMP_SPLIT = 1  # number of leading x batches split into per-quantum DMAs
DELAY_STEAL = False  # emit the stolen D2 one quantum late
XBUFS = 3
RBUFS = 4  # covers the 2-quantum stage offset + ScalarE run-ahead
HBUFS = 3

_BUILD_CACHE: dict = {}
_LAST_IN_MAPS: list | None = None


def _build_program(
    c4: int,
    k_steal: int = None,
    s_tail: int = None,
    delay_steal: bool = None,
    dma_batch: int = None,
    out_batch: int = None,
    xbufs: int = None,
    rbufs: int = None,
    hbufs: int = None,
    s_phase: int = None,
    end_alt: int = None,
    ramp_swap: int = None,
    offset: int = 2,
    steal_quanta: frozenset = None,
):
    """Build + compile the 8-core SPMD Bass program for C4 partial columns."""
    import concourse.tile as tile
    from concourse import bacc, mybir

    k_steal = K_STEAL if k_steal is None else k_steal
    s_tail = S_TAIL if s_tail is None else s_tail
    delay_steal = DELAY_STEAL if delay_steal is None else delay_steal
    dma_batch = DMA_BATCH if dma_batch is None else dma_batch
    out_batch = OUT_BATCH if out_batch is None else out_batch
    xbufs = XBUFS if xbufs is None else xbufs
    rbufs = RBUFS if rbufs is None else rbufs
    hbufs = HBUFS if hbufs is None else hbufs
    s_phase = (S_PHASE if s_phase is None else s_phase) % max(k_steal, 1)
    end_alt = END_ALT if end_alt is None else end_alt
    ramp_swap = RAMP_SWAP if ramp_swap is None else ramp_swap
    s_head = 2 * Q_COLS - s_tail

    f32 = mybir.dt.float32
    bf16 = mybir.dt.bfloat16
    f8 = mybir.dt.float8e4
    Relu = mybir.ActivationFunctionType.Relu
    add_op = mybir.AluOpType.add
    max_op = mybir.AluOpType.max

    nq = c4 // Q_COLS
    assert c4 % Q_COLS == 0

    nc = bacc.Bacc(
        "TRN2",
        target_bir_lowering=False,
        debug=False,
        enable_asserts=False,
        num_devices=N_CORES,
    )

    xT = nc.dram_tensor("xT", [128, c4], bf16, kind="ExternalInput").ap()
    w1 = nc.dram_tensor("w1", [128, 128], bf16, kind="ExternalInput").ap()
    w2 = nc.dram_tensor("w2", [128, 128], bf16, kind="ExternalInput").ap()
    b1 = nc.dram_tensor("b1", [128, 1], f32, kind="ExternalInput").ap()
    b2 = nc.dram_tensor("b2", [128, 1], f32, kind="ExternalInput").ap()
    pout = nc.dram_tensor("pout", [128, 2 * c4], f8, kind="ExternalOutput").ap()

    with ExitStack() as ctx:
        tc = ctx.enter_context(tile.TileContext(nc))
        singles = ctx.enter_context(tc.tile_pool(name="singles", bufs=1))
        xpool = ctx.enter_context(tc.tile_pool(name="xc", bufs=xbufs))
        rpool = ctx.enter_context(tc.tile_pool(name="rhs2", bufs=rbufs))
        hpool = ctx.enter_context(tc.tile_pool(name="h2r", bufs=hbufs))
        abpool = ctx.enter_context(tc.tile_pool(name="ab", bufs=2, space="PSUM"))
        cdpool = ctx.enter_context(tc.tile_pool(name="cd", bufs=2, space="PSUM"))

        w1sb = singles.tile([128, 128], bf16)
        w2sb = singles.tile([128, 128], bf16)
        b1sb = singles.tile([128, 1], f32)
        b2sb = singles.tile([128, 1], f32)
        # Ramp: w1+b1 lead the sync/HWDGE queue (tiny transfers, ~100ns each)
        # ahead of the first x chunk; w2+b2 ride the SWDGE ring in parallel
        # (not needed until MM2(0)/D2(0), ~1.5us later).
        nc.sync.dma_start(out=w1sb, in_=w1)
        nc.sync.dma_start(out=b1sb, in_=b1)
        nc.gpsimd.dma_start(out=w2sb, in_=w2)
        nc.gpsimd.dma_start(out=b2sb, in_=b2)

        # Pre-warm the ScalarE activation table (~2.7us PSEUDO_LOAD_ACT_FUNC_SET
        # attaches to the first ACTIVATE) so it overlaps the first x-chunk DMA.
        warm = singles.tile([128, 1], f32)
        nc.vector.memset(warm, 0.0)
        nc.scalar.activation(warm, warm, Relu)
        ab0 = None
        if PE_WARM:
            # Start the PE's p-state ramp clock during the first x-chunk DMA
            # so the first real matmuls run at full clock, not the cold one.
            # The dummy writes into quantum 0's ab tile, which MM1(0) resets
            # anyway (start=True).
            ab0 = abpool.tile([128, 2 * Q_COLS], f32, name="ab")
            nc.tensor.matmul(
                out=ab0[0:1, 0:1],
                lhsT=warm[0:1, 0:1],
                rhs=warm[0:1, 0:1],
                start=True,
                stop=True,
            )

        Q = Q_COLS
        # Steal schedule: every k_steal-th quantum ScalarE takes the D2
        # drain to balance the engines (ACT 1.2 col/ns vs DVE 0.96), and
        # the last END_ALT quanta alternate when D1 work runs out.
        steals = set()
        for p in range(nq):
            s = k_steal > 0 and p % k_steal == s_phase
            if nq - 1 - p < end_alt:
                s = (nq - 1 - p) % 2 == 1
            if steal_quanta is not None:
                s = p in steal_quanta
            if s:
                steals.add(p)
        # Stage-2 processing order.  (A steal-swap variant -- processing
        # the stolen quantum's successor first so its cd tile lands in a
        # long-free buffer -- measured WORSE: it moves the buffer-parity
        # stall from VectorE onto the busier ScalarE stream.)
        order = list(range(nq))

        # Output batches: pairs of quanta per DMA, except the last
        # TAIL_SINGLE quanta go one per DMA so the final transfer is half
        # as long and starts one quantum earlier.
        p2batch = {}
        batch_info = {}  # bid -> (first quantum, bq)
        i = 0
        bid = 0
        while i < nq:
            bq = 1 if i >= nq - TAIL_SINGLE else min(out_batch, nq - i)
            for j in range(bq):
                p2batch[i + j] = (bid, j)
            batch_info[bid] = (i, bq)
            i += bq
            bid += 1

        # Software-pipelined emission, offset 2: stage-2 work is emitted
        # two iterations behind stage-1.  In the PE's in-order stream,
        # MM2(p) then only waits on a long-finished D1, so the PE never
        # stalls and ScalarE's D1 cadence decouples from the MM2/D2 chain.
        xc = None
        rhs2s = {}
        h2r_tiles = {}
        h2r_done = {}
        for q in range(nq + offset):
            if q < nq:
                if q % dma_batch == 0:
                    bq = min(dma_batch, nq - q)
                    xc = xpool.tile([128, bq * Q], bf16)
                    if q < RAMP_SPLIT * dma_batch and bq > 1:
                        # Split early batches so MM1 starts after a single
                        # quantum's transfer during the ramp.
                        nc.sync.dma_start(out=xc[:, 0:Q], in_=xT[:, q * Q : (q + 1) * Q])
                        nc.sync.dma_start(
                            out=xc[:, Q : bq * Q],
                            in_=xT[:, (q + 1) * Q : (q + bq) * Q],
                        )
                    else:
                        nc.sync.dma_start(
                            out=xc, in_=xT[:, q * Q : (q + bq) * Q]
                        )
                jx = (q % dma_batch) * Q

                if q == 0 and ab0 is not None:
                    ab = ab0
                else:
                    ab = abpool.tile([128, 2 * Q], f32)
                nc.tensor.matmul(
                    out=ab[:, 0:Q],
                    lhsT=w1sb[0:64, :],
                    rhs=xc[0:64, jx : jx + Q],
                    start=True,
                    stop=True,
                )
                nc.tensor.matmul(
                    out=ab[:, Q : 2 * Q],
                    lhsT=w1sb[64:128, :],
                    rhs=xc[64:128, jx : jx + Q],
                    start=True,
                    stop=True,
                )

                # D1 drain: ScalarE (GpSimd and DMA cannot touch PSUM on
                # TRN2; only ACT at 1.2 col/ns and DVE at 0.96 can).
                rhs2 = rpool.tile([128, 2 * Q], bf16)
                rhs2s[q] = rhs2
                if q < ramp_swap and q % 2 == 1:
                    nc.vector.tensor_scalar(
                        rhs2, ab, b1sb, 0.0, add_op, max_op
                    )
                else:
                    nc.scalar.activation(rhs2, ab, Relu, bias=b1sb)

            k2 = q - offset  # stage-2 slot
            if k2 < 0 or k2 >= nq:
                continue
            p = order[k2]
            rhs2 = rhs2s.pop(p)
            b, boff = p2batch[p]
            if b not in h2r_tiles:
                bq = batch_info[b][1]
                h2r_tiles[b] = (
                    hpool.tile([128, bq * 2 * Q], f8, name="h2r"),
                    bq,
                )
                h2r_done[b] = 0
            h2r, bq = h2r_tiles[b]
            jh = boff * 2 * Q

            cd = cdpool.tile([128, 2 * Q], f32)
            nc.tensor.matmul(
                out=cd[:, 0:Q], lhsT=w2sb, rhs=rhs2[:, 0:Q], start=True, stop=True
            )
            nc.tensor.matmul(
                out=cd[:, Q : 2 * Q],
                lhsT=w2sb,
                rhs=rhs2[:, Q : 2 * Q],
                start=True,
                stop=True,
            )

            dst = h2r[:, jh : jh + 2 * Q]
            steal = p in steals
            if steal and s_head > 0:
                nc.vector.tensor_scalar(
                    dst[:, 0:s_head], cd[:, 0:s_head], b2sb, 0.0, add_op, max_op
                )
            elif not steal:
                if p - 1 in steals and POST_SPLIT:
                    # Post-steal quantum: VectorE resumes gated on this
                    # quantum's MM2 (its cd buffer was busy until the
                    # pre-steal drain finished).  Draining in halves lets
                    # it start after MM2a alone, ~350ns earlier, at the
                    # cost of one extra access-latency bubble.
                    nc.vector.tensor_scalar(
                        dst[:, 0:Q], cd[:, 0:Q], b2sb, 0.0, add_op, max_op
                    )
                    nc.vector.tensor_scalar(
                        dst[:, Q : 2 * Q],
                        cd[:, Q : 2 * Q],
                        b2sb,
                        0.0,
                        add_op,
                        max_op,
                    )
                else:
                    nc.vector.tensor_scalar(dst, cd, b2sb, 0.0, add_op, max_op)
            if steal:
                nc.scalar.activation(
                    dst[:, s_head : 2 * Q], cd[:, s_head : 2 * Q], Relu, bias=b2sb
                )

            h2r_done[b] += 1
            if h2r_done[b] == bq:
                c0 = 2 * batch_info[b][0] * Q
                nc.sync.dma_start(
                    out=pout[:, c0 : c0 + bq * 2 * Q], in_=h2r
                )
                del h2r_tiles[b]

    nc.compile()
    return nc


def _get_program(c4: int):
    if c4 not in _BUILD_CACHE:
        _BUILD_CACHE[c4] = _build_program(c4)
    return _BUILD_CACHE[c4]


def kernel(x, batch, num_graphs, W_in, b_in, W_h, b_h, W_out, b_out):
    from concourse import bass_utils

    x = np.asarray(x, dtype=np.float32)
    batch = np.asarray(batch).astype(np.int64)
    g_total = int(num_graphs)
    W_in = np.asarray(W_in, dtype=np.float32)
    b_in = np.asarray(b_in, dtype=np.float32)
    W_h = np.asarray(W_h, dtype=np.float32)
    b_h = np.asarray(b_h, dtype=np.float32)
    W_out = np.asarray(W_out, dtype=np.float32)
    b_out = np.asarray(b_out, dtype=np.float32)

    if batch.size and np.any(np.diff(batch) < 0):
        order = np.argsort(batch, kind="stable")
        x = x[order]
        batch = batch[order]

    n_nodes, f_in = x.shape
    h_dim = W_in.shape[1]
    assert f_in == F_IN and h_dim == H_DIM
    assert g_total % N_CORES == 0
    g_per_core = g_total // N_CORES

    counts = np.bincount(batch, minlength=g_total).astype(np.int64)
    node_starts = np.concatenate([[0], np.cumsum(counts)])  # [G+1]

    # Per-graph padded counts (multiple of PACK).
    pc_counts = (counts + PACK - 1) // PACK * PACK

    # Per-core geometry.
    core_g0 = [c * g_per_core for c in range(N_CORES)]
    core_pad_tot = [
        int(pc_counts[c * g_per_core : (c + 1) * g_per_core].sum())
        for c in range(N_CORES)
    ]
    c4_per_core = [t // PACK for t in core_pad_tot]
    align = 2 * Q_COLS  # even quantum count
    c4 = max(c4_per_core)
    c4 = (c4 + align - 1) // align * align  # uniform, quantum aligned

    # Constant tensors shared by all cores.
    w1blk = np.zeros((128, 128), dtype=np.float32)
    w1blk[0:32, 0:64] = W_in
    w1blk[32:64, 64:128] = W_in
    w1blk[64:96, 0:64] = W_in
    w1blk[96:128, 64:128] = W_in
    w1blk = w1blk.astype(ml_dtypes.bfloat16)
    w2blk = np.zeros((128, 128), dtype=np.float32)
    w2blk[0:64, 0:64] = W_h
    w2blk[64:128, 64:128] = W_h
    w2blk = w2blk.astype(ml_dtypes.bfloat16)
    b1cat = np.tile(b_in, 2).reshape(128, 1).astype(np.float32)
    b2cat = np.tile(b_h, 2).reshape(128, 1).astype(np.float32)

    # Per-core packed inputs.
    in_maps = []
    for c in range(N_CORES):
        g0 = core_g0[c]
        g1 = g0 + g_per_core
        s, e = int(node_starts[g0]), int(node_starts[g1])
        pc_c = pc_counts[g0:g1]
        pad_starts = np.concatenate([[0], np.cumsum(pc_c)])  # [g_per_core+1]

        x_padded = np.zeros((c4 * PACK, f_in), dtype=np.float32)
        if e > s:
            local_batch = batch[s:e] - g0
            # dst = pad_start of graph + index within graph
            dst = pad_starts[local_batch] + (
                np.arange(s, e) - node_starts[g0 + local_batch]
            )
            x_padded[dst] = x[s:e]
        xT_dev = (
            x_padded.reshape(c4, PACK, f_in).transpose(1, 2, 0).reshape(128, c4)
        )
        xT_dev = np.ascontiguousarray(xT_dev).astype(ml_dtypes.bfloat16)
        in_maps.append(
            {
                "xT": xT_dev,
                "w1": w1blk,
                "w2": w2blk,
                "b1": b1cat,
                "b2": b2cat,
            }
        )

    global _LAST_IN_MAPS
    _LAST_IN_MAPS = in_maps

    nc = _get_program(c4)
    res = bass_utils.run_bass_kernel_spmd(
        nc, in_maps, core_ids=list(range(N_CORES))
    )

    # Pad-node contribution, exactly as the device computes it for x=0 rows:
    # h1raw = 0 (fp32 psum) -> D1: bf16(relu(b1)) -> S2 bf16 matmul (fp32 acc)
    # -> D2: fp8e4(relu(. + b2)).
    bf = ml_dtypes.bfloat16
    f8 = ml_dtypes.float8_e4m3
    h1p = np.maximum(b_in, 0.0).astype(bf).astype(np.float32)
    w2bf = W_h.astype(bf).astype(np.float32)
    vpad = np.maximum(h1p @ w2bf + b_h, 0.0).astype(f8).astype(np.float64)

    out = np.zeros((g_total, W_out.shape[1]), dtype=np.float32)
    for c in range(N_CORES):
        g0 = core_g0[c]
        g1 = g0 + g_per_core
        cnt_c = counts[g0:g1].astype(np.float64)
        pc_c = pc_counts[g0:g1]
        pad_starts = np.concatenate([[0], np.cumsum(pc_c)])
        col_starts = pad_starts // PACK  # [g_per_core+1]

        nq = c4 // Q_COLS
        P = np.asarray(res.results[c]["pout"]).astype(np.float32)
        # unfolded layout: [128, nq, 2 streams, Q]; fold partitions halves
        # (node pairs) and streams (node quads)
        Pr = P.reshape(128, nq, 2, Q_COLS)
        R1 = (Pr[0:64] + Pr[64:128]).sum(axis=2).reshape(64, c4)
        cs = np.concatenate(
            [np.zeros((64, 1)), np.cumsum(R1.astype(np.float64), axis=1)], axis=1
        )  # [64, c4+1]
        seg_sum = (cs[:, col_starts[1:]] - cs[:, col_starts[:-1]]).T  # [g, 64]

        n_pad = (pc_c - counts[g0:g1]).astype(np.float64)
        seg_sum = seg_sum - n_pad[:, None] * vpad[None, :]
        denom = np.maximum(cnt_c, 1.0)
        mean = seg_sum / denom[:, None]
        mean[cnt_c == 0] = 0.0
        out[g0:g1] = mean.astype(np.float32) @ W_out + b_out

    return out
